# revision 47
# baseline (speedup 1.0000x reference)
"""Trainium2 Bass kernel for nn_CausalSelfAttention_42176578846970.

Sliding-window causal self-attention with paired heads, RoPE + QK RMSNorm,
value-embedding gate and per-head attention gate.

Sharding: 8 cores = 2 batch groups x 4-way tensor parallel over the 8 paired
heads (2 paired heads / 4 original heads = 512 channels per core). Each core
computes its partial output projection (transposed, [E, T]); a ReduceScatter
over each 4-core batch group sums the partials and leaves each core with 512
embed-rows of out.T, which the host transposes and concatenates.

All shapes/strategy hardcoded for B=2, T=2048, E=2048, H=16, hd=128, W=1024.
"""

import sys

for _p in ("/opt/trn_rl_repo", "/root/.axon_site/_ro/trn_rl_repo"):
    if _p not in sys.path:
        sys.path.insert(0, _p)

import numpy as np
import ml_dtypes

import concourse.bass as bass
import concourse.bacc as bacc
import concourse.mybir as mybir
import concourse.tile as tile
from concourse.bass_utils import run_bass_kernel_spmd

BF16 = mybir.dt.bfloat16
F32 = mybir.dt.float32
NPBF16 = ml_dtypes.bfloat16

T = 2048          # tokens
E = 2048          # embed dim
C = 512           # channels per core (4 heads x 128)
HD = 128          # head dim
NHL = 4           # local heads
W = 1024          # window
TT = T // 128     # 16 token tiles
ET = E // 128     # 16 embed tiles
QB = 4            # q blocks of 512
EPS = 1.1920929e-07
MASK_NEG = -1.0e5
SCALE = 1.0 / 16.0  # 1/sqrt(2*HD)

_CACHE = {}


def _build_nc():
    nc = bacc.Bacc("TRN2", target_bir_lowering=False, debug=False, num_devices=8)

    # ---- DRAM parameters (per-core shards) ----
    # xT is padded with a duplicate of token 0 at column 0, so the shifted
    # (previous-token) read for the second key half never goes out of range.
    xT = nc.declare_dram_parameter("xT", [E, T + 1], BF16, isOutput=False)
    wqt = nc.declare_dram_parameter("wqt", [E, C], BF16, isOutput=False)
    wkt = nc.declare_dram_parameter("wkt", [E, C], BF16, isOutput=False)
    wvt = nc.declare_dram_parameter("wvt", [E, C], BF16, isOutput=False)
    wot = nc.declare_dram_parameter("wot", [C, E], BF16, isOutput=False)
    ve2 = nc.declare_dram_parameter("ve2", [T, C], BF16, isOutput=False)
    cosr = nc.declare_dram_parameter("cosr", [128, T], BF16, isOutput=False)
    sinr = nc.declare_dram_parameter("sinr", [128, T], BF16, isOutput=False)
    vegw = nc.declare_dram_parameter("vegw", [32, NHL], BF16, isOutput=False)
    agw = nc.declare_dram_parameter("agw", [12, NHL], BF16, isOutput=False)
    cps = nc.declare_dram_parameter("cps", [1, 1], F32, isOutput=False)
    m0 = nc.declare_dram_parameter("m0", [128, 128], F32, isOutput=False)
    m1024 = nc.declare_dram_parameter("m1024", [128, 128], F32, isOutput=False)
    out_ext = nc.declare_dram_parameter("out", [C, T], BF16, isOutput=True)

    with tile.TileContext(nc) as tc:
        with (
            tc.tile_pool(name="dram", bufs=1, space="DRAM") as dramp,
            tc.tile_pool(name="consts", bufs=1) as consts,
            tc.tile_pool(name="persist", bufs=1) as persist,
            tc.tile_pool(name="xtp", bufs=2) as xtp,
            tc.tile_pool(name="vep", bufs=2) as vep,
            tc.tile_pool(name="work", bufs=2) as work,
            tc.tile_pool(name="attnp", bufs=3) as attnp,
            tc.tile_pool(name="ps_mm", bufs=6, space="PSUM") as ps_mm,
            tc.tile_pool(name="ps_sm", bufs=2, space="PSUM") as ps_sm,
        ):
            outT_dram = [
                dramp.tile([E, 512], BF16, tag=f"outT{j}", name=f"outT{j}")
                for j in range(QB)
            ]
            rs_dram = [
                dramp.tile([C, 512], BF16, tag=f"rs{j}", name=f"rs{j}")
                for j in range(QB)
            ]

            # ---- load constants ----
            wq_sb = consts.tile([128, ET, C], BF16, tag="wq")
            wk_sb = consts.tile([128, ET, C], BF16, tag="wk")
            wv_sb = consts.tile([128, ET, C], BF16, tag="wv")
            for c4 in range(4):
                sl = slice(4 * c4, 4 * c4 + 4)
                nc.gpsimd.dma_start(
                    wq_sb[:, sl, :],
                    wqt[:].rearrange("(a p) c -> p a c", p=128)[:, sl, :])
                nc.gpsimd.dma_start(
                    wk_sb[:, sl, :],
                    wkt[:].rearrange("(a p) c -> p a c", p=128)[:, sl, :])
                nc.gpsimd.dma_start(
                    wv_sb[:, sl, :],
                    wvt[:].rearrange("(a p) c -> p a c", p=128)[:, sl, :])
            wo_sb = consts.tile([128, 4, E], BF16, tag="wo")
            nc.gpsimd.dma_start(wo_sb, wot[:].rearrange("(a p) e -> p a e", p=128))
            cos_sb = consts.tile([128, T], BF16, tag="cos")
            sin_sb = consts.tile([128, T], BF16, tag="sin")
            nc.gpsimd.dma_start(cos_sb, cosr[:])
            nc.gpsimd.dma_start(sin_sb, sinr[:])
            sel0 = consts.tile([128, 1], BF16, tag="sel0")
            sel1 = consts.tile([128, 1], BF16, tag="sel1")
            nc.vector.memset(sel0, 0.0)
            nc.vector.memset(sel0[0:64, :], 1.0)
            nc.vector.memset(sel1, 0.0)
            nc.vector.memset(sel1[64:128, :], 1.0)
            vegw_sb = consts.tile([32, NHL], BF16, tag="vegw")
            agw_sb = consts.tile([12, NHL], BF16, tag="agw")
            nc.gpsimd.dma_start(vegw_sb, vegw[:])
            nc.gpsimd.dma_start(agw_sb, agw[:])
            m0_sb = consts.tile([128, 128], F32, tag="m0")
            m1024_sb = consts.tile([128, 128], F32, tag="m1024")
            nc.gpsimd.dma_start(m0_sb, m0[:])
            nc.gpsimd.dma_start(m1024_sb, m1024[:])
            cps_sb = consts.tile([1, 1], F32, tag="cps")
            nc.gpsimd.dma_start(cps_sb, cps[:])
            cps1 = consts.tile([1, 1], F32, tag="cps1")
            nc.vector.tensor_scalar_add(cps1, cps_sb, 1.0)

            eps_sb = consts.tile([128, 1], F32, tag="eps")
            nc.vector.memset(eps_sb, EPS)

            ones_col = consts.tile([128, 1], BF16, tag="ones_col")
            nc.vector.memset(ones_col, 1.0)
            ones_row = consts.tile([1, 128], BF16, tag="ones_row")
            nc.vector.memset(ones_row, 1.0)

            # ---- persistent intermediates ----
            v_sb = persist.tile([128, TT * C], BF16, tag="v")
            kT_sb = persist.tile([128, 4 * T], BF16, tag="kT")
            agate_sb = persist.tile([1, NHL * T], BF16, tag="agate")

            xT_r = xT[:].rearrange("(a p) t -> p a t", p=128)

            # ===== projections (q,k channel-major; v token-major) ========
            # q/k are produced directly channel-major (pair tiles: x1 halves
            # in c-tile p, x2 halves in c-tile p+2), roped and rms-normed via
            # selector-matmul partition sums + rank-1 broadcast.
            for tb in range(QB):
                tb0 = tb * 512
                xt = xtp.tile([128, ET, 513], BF16, tag="xt")
                # cols [tb0, tb0+513) of padded xT = tokens [tb0-1, tb0+512)
                # chunked so the first e-tiles land before the rest arrive
                for c4 in range(4):
                    sl = slice(4 * c4, 4 * c4 + 4)
                    nc.sync.dma_start(
                        xt[:, sl, :], xT_r[:, sl, tb0 : tb0 + 513])
                cosb = cos_sb[:, tb0 : tb0 + 512]
                sinb = sin_sb[:, tb0 : tb0 + 512]

                qt = work.tile([128, 4 * 512], BF16, tag="qt", bufs=1)
                # phase A: projection matmuls + rope + squares for all four
                # (tensor, pair) units; phase B: the norm/apply chains.
                # Splitting keeps the (in-order) PE stream dense: unit u+1's
                # projection matmuls are emitted before unit u's norm matmuls.
                units = [(w, p) for w in ("q", "k") for p in range(2)]
                rAs, rBs, sqAs, sqBs = {}, {}, {}, {}
                for which, p in units:
                    w_sb = wq_sb if which == "q" else wk_sb
                    psA = ps_mm.tile([128, 512], F32, tag="mm", name=f"psA{which}{tb}{p}")
                    psB = ps_mm.tile([128, 512], F32, tag="mm", name=f"psB{which}{tb}{p}")
                    shift = 1 if which == "q" else 0
                    for ie in range(ET):
                        st = ie == 0
                        sp = ie == ET - 1
                        nc.tensor.matmul(
                            psA, w_sb[:, ie, p * 128 : (p + 1) * 128],
                            xt[:, ie, 1:513], start=st, stop=sp)
                        nc.tensor.matmul(
                            psB, w_sb[:, ie, (p + 2) * 128 : (p + 3) * 128],
                            xt[:, ie, shift : shift + 512], start=st, stop=sp)
                    cA = work.tile([128, 512], BF16, tag="cA")
                    cB = work.tile([128, 512], BF16, tag="cB")
                    nc.scalar.activation(cA, psA, mybir.ActivationFunctionType.Copy)
                    nc.scalar.activation(cB, psB, mybir.ActivationFunctionType.Copy)
                    rA = work.tile([128, 512], BF16, tag="rA", bufs=4,
                                   name=f"rA{which}{tb}{p}")
                    rB = work.tile([128, 512], BF16, tag="rB", bufs=4,
                                   name=f"rB{which}{tb}{p}")
                    tmp = work.tile([128, 512], BF16, tag="tmp")
                    tmp2 = work.tile([128, 512], BF16, tag="tmp", name="tmp2")
                    nc.vector.tensor_mul(rA, cA, cosb)
                    nc.vector.tensor_mul(tmp, cB, sinb)
                    nc.vector.tensor_add(rA, rA, tmp)
                    nc.vector.tensor_mul(rB, cB, cosb)
                    nc.vector.tensor_mul(tmp2, cA, sinb)
                    nc.vector.tensor_sub(rB, rB, tmp2)
                    sqA = work.tile([128, 512], BF16, tag="sqA", bufs=3,
                                    name=f"sqA{which}{tb}{p}")
                    sqB = work.tile([128, 512], BF16, tag="sqB", bufs=3,
                                    name=f"sqB{which}{tb}{p}")
                    nc.vector.tensor_mul(sqA, rA, rA)
                    nc.vector.tensor_mul(sqB, rB, rB)
                    key = (which, p)
                    rAs[key], rBs[key] = rA, rB
                    sqAs[key], sqBs[key] = sqA, sqB

                for which, p in units:
                    rA, rB = rAs[(which, p)], rBs[(which, p)]
                    sqA, sqB = sqAs[(which, p)], sqBs[(which, p)]
                    # rms-norm per head: head 2p = partitions 0:64 of A,B;
                    # head 2p+1 = partitions 64:128
                    psn0 = ps_sm.tile([1, 512], F32, tag="sm")
                    nc.tensor.matmul(psn0, sel0, sqA, start=True, stop=False)
                    nc.tensor.matmul(psn0, sel0, sqB, start=False, stop=True)
                    psn1 = ps_sm.tile([1, 512], F32, tag="sm")
                    nc.tensor.matmul(psn1, sel1, sqA, start=True, stop=False)
                    nc.tensor.matmul(psn1, sel1, sqB, start=False, stop=True)
                    rms0 = work.tile([1, 512], F32, tag="rms0")
                    rms1 = work.tile([1, 512], F32, tag="rms0", name="rms1")
                    nc.scalar.activation(
                        rms0, psn0, mybir.ActivationFunctionType.Sqrt,
                        bias=eps_sb[0:1, 0:1], scale=1.0 / HD)
                    nc.scalar.activation(
                        rms1, psn1, mybir.ActivationFunctionType.Sqrt,
                        bias=eps_sb[0:1, 0:1], scale=1.0 / HD)
                    ri0 = work.tile([1, 512], BF16, tag="ri0")
                    ri1 = work.tile([1, 512], BF16, tag="ri0", name="ri1")
                    with nc.allow_low_precision(
                        reason="bf16 rms scale is well within tolerance"
                    ):
                        nc.vector.reciprocal(ri0, rms0)
                        nc.vector.reciprocal(ri1, rms1)
                    psb = ps_mm.tile([128, 512], F32, tag="mm", name=f"psb{which}{tb}{p}")
                    nc.tensor.matmul(
                        psb[0:64, :], ones_row[0:1, 0:64], ri0,
                        start=True, stop=True, skip_group_check=True)
                    nc.tensor.matmul(
                        psb[64:128, :], ones_row[0:1, 0:64], ri1,
                        start=True, stop=True, skip_group_check=True)
                    if which == "q":
                        nc.vector.tensor_mul(
                            qt[:, p * 512 : (p + 1) * 512], rA, psb)
                        nc.vector.tensor_mul(
                            qt[:, (p + 2) * 512 : (p + 3) * 512], rB, psb)
                    else:
                        nc.vector.tensor_mul(
                            kT_sb[:, p * T + tb0 : p * T + tb0 + 512], rA, psb)
                        nc.vector.tensor_mul(
                            kT_sb[:, (p + 2) * T + tb0 : (p + 2) * T + tb0 + 512],
                            rB, psb)

                # v (token-major) + ve gate
                for tt in range(4):
                    it = tb * 4 + tt
                    toff = 1 + tt * 128
                    vet = vep.tile([128, C], BF16, tag="ve")
                    nc.sync.dma_start(vet, ve2[tb0 + tt * 128 : tb0 + tt * 128 + 128, :])
                    psv = ps_mm.tile([128, C], F32, tag="mm")
                    for ie in range(ET):
                        nc.tensor.matmul(
                            psv, xt[:, ie, toff : toff + 128], wv_sb[:, ie, :],
                            start=(ie == 0), stop=(ie == ET - 1))
                    psg = ps_sm.tile([128, NHL], F32, tag="sm")
                    nc.tensor.matmul(
                        psg, xt[0:32, 0, toff : toff + 128], vegw_sb,
                        start=True, stop=True)
                    gate = work.tile([128, NHL], F32, tag="gate")
                    nc.scalar.activation(
                        gate, psg, mybir.ActivationFunctionType.Sigmoid)
                    for h in range(NHL):
                        hs = slice(h * HD, (h + 1) * HD)
                        nc.vector.scalar_tensor_tensor(
                            out=v_sb[:, it * C + h * HD : it * C + (h + 1) * HD],
                            in0=vet[:, hs],
                            scalar=gate[:, h : h + 1],
                            in1=psv[:, hs],
                            op0=mybir.AluOpType.mult,
                            op1=mybir.AluOpType.add,
                        )

                # attention gate rows (channel-major, one row per head)
                for h in range(NHL):
                    psa = ps_sm.tile([1, 512], F32, tag="sm")
                    nc.tensor.matmul(
                        psa, agw_sb[:, h : h + 1], xt[0:12, 0, 1:513],
                        start=True, stop=True)
                    nc.scalar.activation(
                        agate_sb[0:1, h * T + tb0 : h * T + tb0 + 512], psa,
                        mybir.ActivationFunctionType.Sigmoid)

                # ---- attention for this q-block (fused with projection) --
                qb = tb
                q0 = tb0
                yt = work.tile([128, 4 * 512], BF16, tag="yt")
                for m in range(2):  # local paired heads
                    cts = (m, m + 2)
                    kt_lo = max(0, (q0 - W) // 128)
                    kt_hi = q0 // 128 + 3
                    kts = list(range(kt_lo, kt_hi + 1))
                    psy = [
                        ps_mm.tile([128, 512], F32, tag="mm", name=f"psy{m}_{qb}_{ci}")
                        for ci in range(2)
                    ]
                    psr = ps_sm.tile([1, 512], F32, tag="sm")
                    for ki, kt in enumerate(kts):
                        k0 = kt * 128
                        # full 512-wide block; invalid regions masked below
                        pss = ps_mm.tile([128, 512], F32, tag="mm")
                        for ci, ct in enumerate(cts):
                            nc.tensor.matmul(
                                pss,
                                kT_sb[:, ct * T + k0 : ct * T + k0 + 128],
                                qt[:, ct * 512 : (ct + 1) * 512],
                                start=(ci == 0), stop=(ci == 1),
                            )
                        # causal: cols q < k0 are fully masked
                        if k0 > q0:
                            nc.vector.tensor_scalar_add(
                                pss[:, 0 : k0 - q0], pss[:, 0 : k0 - q0], MASK_NEG
                            )
                        # causal diagonal sub-block
                        if q0 <= k0 < q0 + 512:
                            off = k0 - q0
                            nc.vector.tensor_add(
                                pss[:, off : off + 128],
                                pss[:, off : off + 128], m0_sb,
                            )
                        # window diagonal sub-block (q - k == W boundary)
                        if q0 <= k0 + W < q0 + 512:
                            off = k0 + W - q0
                            nc.vector.tensor_add(
                                pss[:, off : off + 128],
                                pss[:, off : off + 128], m1024_sb,
                            )
                        # beyond window: cols q >= k0 + W + 128 fully masked
                        hi = k0 + W + 128 - q0
                        if hi < 512:
                            nc.vector.tensor_scalar_add(
                                pss[:, hi:512], pss[:, hi:512], MASK_NEG
                            )
                        at = attnp.tile([128, 512], BF16, tag="at")
                        nc.scalar.activation(
                            at, pss,
                            mybir.ActivationFunctionType.Exp, scale=SCALE,
                        )
                        first = ki == 0
                        last = ki == len(kts) - 1
                        nc.tensor.matmul(
                            psr, ones_col, at,
                            start=first, stop=last, skip_group_check=True,
                        )
                        for ci in range(2):
                            nc.tensor.matmul(
                                psy[ci],
                                v_sb[:, kt * C + (2 * m + ci) * HD : kt * C + (2 * m + ci + 1) * HD],
                                at,
                                start=first, stop=last, skip_group_check=True,
                            )
                    rv = work.tile([1, 512], F32, tag="rv")
                    nc.vector.reciprocal(rv, psr)
                    for ci in range(2):
                        h = 2 * m + ci
                        fh = work.tile([1, 512], BF16, tag="fh")
                        nc.vector.scalar_tensor_tensor(
                            out=fh,
                            in0=agate_sb[0:1, h * T + q0 : h * T + q0 + 512],
                            scalar=cps1,
                            in1=rv,
                            op0=mybir.AluOpType.mult,
                            op1=mybir.AluOpType.mult,
                        )
                        psf = ps_mm.tile([128, 512], F32, tag="mm")
                        nc.tensor.matmul(psf, ones_row, fh, start=True, stop=True)
                        fsb = work.tile([128, 512], BF16, tag="fsb")
                        nc.scalar.activation(
                            fsb, psf, mybir.ActivationFunctionType.Copy
                        )
                        nc.vector.tensor_mul(
                            yt[:, h * 512 : (h + 1) * 512], psy[ci], fsb
                        )

                # ---- output projection for this q-block ----
                for et in range(ET):
                    e0 = et * 128
                    pso = ps_mm.tile([128, 512], F32, tag="mm")
                    for ct in range(4):
                        nc.tensor.matmul(
                            pso,
                            wo_sb[:, ct, e0 : e0 + 128],
                            yt[:, ct * 512 : (ct + 1) * 512],
                            start=(ct == 0), stop=(ct == 3),
                        )
                    osb = work.tile([128, 512], BF16, tag="osb")
                    if et % 2 == 0:
                        nc.vector.tensor_copy(osb, pso)
                    else:
                        nc.scalar.activation(
                            osb, pso, mybir.ActivationFunctionType.Copy
                        )
                    nc.sync.dma_start(outT_dram[qb][e0 : e0 + 128, :], osb)

                # ---- reduce-scatter this q-block's partial output ----
                nc.gpsimd.collective_compute(
                    "ReduceScatter",
                    mybir.AluOpType.add,
                    replica_groups=[[0, 1, 2, 3], [4, 5, 6, 7]],
                    ins=[outT_dram[qb]],
                    outs=[rs_dram[qb]],
                )
                nc.gpsimd.dma_start(out_ext[:, q0 : q0 + 512], rs_dram[qb])

    nc.compile()
    return nc


def _host_prep(x, ve, cos, sin, Wq, Wk, Wv, Wo, c_proj_scalar, ve_gate_W,
               attn_gate_W):
    """Build the 8 per-core input maps."""
    x = np.asarray(x, np.float32)
    ve = np.asarray(ve, np.float32)
    cos2 = np.asarray(cos, np.float32).reshape(T, 64)
    sin2 = np.asarray(sin, np.float32).reshape(T, 64)
    Wq = np.asarray(Wq, np.float32)
    Wk = np.asarray(Wk, np.float32)
    Wv = np.asarray(Wv, np.float32)
    Wo = np.asarray(Wo, np.float32)
    cpsv = np.asarray(c_proj_scalar, np.float32).reshape(1, 1)
    vgw = np.asarray(ve_gate_W, np.float32)
    agw = np.asarray(attn_gate_W, np.float32)

    # permutation: x1 halves of the 4 local heads first, then x2 halves
    perm = [h * HD + k for h in range(NHL) for k in range(64)] + [
        h * HD + 64 + k for h in range(NHL) for k in range(64)
    ]
    perm = np.asarray(perm)

    cosr = np.ascontiguousarray(np.tile(cos2.T, (2, 1))).astype(NPBF16)
    sinr = np.ascontiguousarray(np.tile(sin2.T, (2, 1))).astype(NPBF16)

    fidx = np.arange(128)
    m0 = np.where(fidx[None, :] >= fidx[:, None], 0.0, MASK_NEG).astype(np.float32)
    m1024 = np.where(fidx[None, :] <= fidx[:, None], 0.0, MASK_NEG).astype(np.float32)

    in_maps = []
    for core in range(8):
        b, g = divmod(core, 4)
        sl = slice(g * C, (g + 1) * C)
        in_maps.append({
            "xT": np.ascontiguousarray(
                np.concatenate([x[b].T[:, :1], x[b].T], axis=1)
            ).astype(NPBF16),
            "wqt": np.ascontiguousarray(Wq[sl][perm].T).astype(NPBF16),
            "wkt": np.ascontiguousarray(Wk[sl][perm].T).astype(NPBF16),
            "wvt": np.ascontiguousarray(Wv[sl].T).astype(NPBF16),
            "wot": np.ascontiguousarray(Wo[:, sl].T).astype(NPBF16),
            "ve2": (2.0 * ve[b, :, sl]).astype(NPBF16),
            "cosr": cosr,
            "sinr": sinr,
            "vegw": np.ascontiguousarray(vgw[4 * g : 4 * g + 4].T).astype(NPBF16),
            "agw": np.ascontiguousarray(agw[4 * g : 4 * g + 4].T).astype(NPBF16),
            "cps": cpsv,
            "m0": m0,
            "m1024": m1024,
        })
    return in_maps


def _run(in_maps, trace=False, **kw):
    if "nc" not in _CACHE:
        _CACHE["nc"] = _build_nc()
    return run_bass_kernel_spmd(
        _CACHE["nc"], in_maps, core_ids=list(range(8)), trace=trace, **kw
    )


def kernel(x, ve, cos, sin, Wq, Wk, Wv, Wo, c_proj_scalar, ve_gate_W,
           attn_gate_W, window_size=1024, **_ignored):
    assert int(window_size) == W, f"kernel hardcodes window={W}"
    in_maps = _host_prep(x, ve, cos, sin, Wq, Wk, Wv, Wo, c_proj_scalar,
                         ve_gate_W, attn_gate_W)
    res = _run(in_maps).results
    outs = []
    for b in range(2):
        outT = np.concatenate(
            [np.asarray(res[4 * b + j]["out"]).astype(np.float32)
             for j in range(4)], axis=0)
        outs.append(outT.T)
    return np.stack(outs).astype(np.float32)
